# revision 6
# baseline (speedup 1.0000x reference)
"""Trainium2 Bass kernel for BinaryLinearWscales — transpose-free v2.

Math:  out = x @ (wscale * sign(weight) + wbias).T
     = x @ w''.T   with   w''[n,k] = wscale_n * sign(weight[n,k]) + wbias_n

Key ideas vs the v1 kernel:
  1. **No on-device transposes.**  The host passes x.T and weight.T
     (layout prep is part of the sharding step), so both matmul operands
     arrive in DRAM already in [K, *] layout.  v1 spent ~1024 PE
     transpose-mode ops (~275 ns each in-context, and transpose-mode
     does not count as PE-busy for the HAM clock gate) interleaved with
     its matmuls.
  2. **Scale and bias folded into the binary weight on-device** (w'' =
     wscale*sign(w) + wbias, computed once on DVE): no xsum
     ones-matmuls, no epilogue math — the PSUM result IS the output.
  3. **bf16 end-to-end.**  Host casts x.T / weight.T to bf16: halves HBM
     traffic (per-core DMA 44 MB vs 76 MB) and keeps the PE at
     1 col/cycle.  Measured rel err 3.2e-3 vs the 2e-2 gate.
  4. **Dense back-to-back matmul stream** (1024 MMs of N=512, nothing
     else on the PE) keeps HAM at K=8/8 (2.4 GHz).  Roofline: 1024 x
     512 cyc / 2.4 GHz = 218 us PE; DMA 44 MB / ~360 GB/s = 122 us.
     TimelineSim predicts 244.5 us/core.
  5. **Few, large DMAs**: x streams in 8 slabs of [4096k x 512t] bf16
     (4 MB), each as 4 batched 1 MB dma_starts via a 3D access pattern
     (p, kc, t) so a single InstDMACopy spans all 16 SDMA engines.
     x slabs ride the SP HWDGE ring; weights + outputs ride the ACT ring.

Sharding (tensor-parallel over DOUT): each of the 8 cores gets 512 rows
of weight/wscale/wbias and the full x; host concatenates core outputs
along the feature dim.

`reps`: number of back-to-back copies of the whole body inside one NEFF
— used by test.py to measure steady-state per-exec device time with the
axon dispatch round-trip cancelled ((t_reps - t_1)/(reps - 1)).
"""

import os
from contextlib import ExitStack

import numpy as np

P = 128

# full problem dims
B, S, DIN, DOUT = 2, 2048, 4096, 4096
N_CORES = 8
N_SHARD = DOUT // N_CORES  # 512

TSLAB = 512  # tokens per x slab


def make_pools(ctx, tc):
    return {
        "x": ctx.enter_context(tc.tile_pool(name="x", bufs=2)),
        "w": ctx.enter_context(tc.tile_pool(name="w", bufs=2)),
        "const": ctx.enter_context(tc.tile_pool(name="const", bufs=1)),
        "osb": ctx.enter_context(tc.tile_pool(name="osb", bufs=4)),
        "pox": ctx.enter_context(tc.tile_pool(name="pox", bufs=4, space="PSUM")),
    }


def build_body(pools, tc, out_ap, xT_ap, wT_ap, wscale_ap, wbias_ap,
               mode="bf16", pfx=""):
    import concourse.bass as bass
    from concourse import mybir
    from concourse.bass import ts

    nc = tc.nc
    K, T = xT_ap.shape
    K2, N = wT_ap.shape
    assert K == K2 and K % P == 0 and T % TSLAB == 0 and N <= 512
    KC = K // P  # 32 k chunks
    NSLAB = T // TSLAB  # 8
    TB = TSLAB // P  # 4

    f32 = mybir.dt.float32
    bf16 = mybir.dt.bfloat16
    f32r = mybir.dt.float32r
    Alu = mybir.AluOpType
    mm_dt = bf16 if mode == "bf16" else f32r

    xpool, wpool, const, opool, pox = (
        pools["x"], pools["w"], pools["const"], pools["osb"], pools["pox"],
    )

    xT3 = xT_ap.rearrange("(kc p) t -> p kc t", p=P)  # [128, KC, T]

    def load_slab(si, split):
        """One x slab = [K, TSLAB] tokens, flat SBUF layout [p, kc*TSLAB+t].

        Batched 3D dma_starts (1 MB each) hit near-peak HBM bandwidth and
        span all 16 SDMA engines per transfer."""
        xs = xpool.tile([P, KC * TSLAB], mm_dt, name=f"{pfx}xs{si}",
                        tag="xs", bufs=2)
        xs3 = xs[:].rearrange("p (kc t) -> p kc t", kc=KC)
        step = KC // split
        dma = nc.sync.dma_start if mode == "bf16" else nc.gpsimd.dma_start
        for d in range(split):
            dma(
                xs3[:, d * step:(d + 1) * step, :],
                xT3[:, d * step:(d + 1) * step, ts(si, TSLAB)],
            )
        return xs

    # x slab 0 first in program order, split fine so the PE starts early
    slabs = {0: load_slab(0, 8)}

    # ---------------- constants ----------------
    wsc_stage = const.tile([1, N], f32, name=f"{pfx}wsc_stage", tag="wsc_stage")
    nc.scalar.dma_start(wsc_stage[:], wscale_ap[:, :])
    wbi_stage = const.tile([1, N], f32, name=f"{pfx}wbi_stage", tag="wbi_stage")
    nc.scalar.dma_start(wbi_stage[:], wbias_ap[:, :])
    # mm_dt copies for same-dtype DVE ops (precision loss is negligible:
    # w'' itself is rounded to mm_dt anyway)
    wsc_nar = const.tile([1, N], mm_dt, name=f"{pfx}wsc_nar", tag="wsc_nar")
    nc.vector.tensor_copy(wsc_nar[:], wsc_stage[:])
    wbi_nar = const.tile([1, N], mm_dt, name=f"{pfx}wbi_nar", tag="wbi_nar")
    nc.vector.tensor_copy(wbi_nar[:], wbi_stage[:])
    wscale_rep = const.tile([P, N], mm_dt, name=f"{pfx}wscale_rep",
                            tag="wscale_rep")
    nc.gpsimd.partition_broadcast(wscale_rep[:], wsc_nar[:])
    wbias_rep = const.tile([P, N], mm_dt, name=f"{pfx}wbias_rep",
                           tag="wbias_rep")
    nc.gpsimd.partition_broadcast(wbias_rep[:], wbi_nar[:])

    # ---------------- w'' = wscale*sign(w) + wbias, cached all kernel -------
    # One persistent SBUF tile [128, KC*N]; DMA'd in 1 MB chunks, signed and
    # scaled on DVE in WCHUNK-kc groups so the first matmuls start early.
    wp = wpool.tile([P, KC * N], mm_dt, name=f"{pfx}wp", tag="wp", bufs=2)
    wp3 = wp[:].rearrange("p (kc n) -> p kc n", kc=KC)
    wT3 = wT_ap.rearrange("(kc p) n -> p kc n", p=P)
    WCHUNK = 4  # kc per production chunk
    wdma = nc.scalar.dma_start if mode == "bf16" else nc.gpsimd.dma_start
    for c in range(KC // WCHUNK):
        sl = slice(c * WCHUNK, (c + 1) * WCHUNK)
        wdma(wp3[:, sl, :], wT3[:, sl, :])
    for c in range(KC // WCHUNK):
        seg = wp[:, c * WCHUNK * N:(c + 1) * WCHUNK * N]
        # (w >= 0) * 2 -> {0, 2}
        nc.vector.tensor_scalar(
            out=seg, in0=seg, scalar1=0.0, scalar2=2.0,
            op0=Alu.is_ge, op1=Alu.mult,
        )
        for kc in range(c * WCHUNK, (c + 1) * WCHUNK):
            wk = wp[:, kc * N:(kc + 1) * N]
            # ({0,2} - 1) * wscale -> +-wscale
            nc.vector.scalar_tensor_tensor(
                out=wk, in0=wk, scalar=-1.0, in1=wscale_rep[:],
                op0=Alu.add, op1=Alu.mult,
            )
            # + wbias
            nc.vector.tensor_add(wk, wk, wbias_rep[:])

    # ---------------- main phase: pure matmul stream ----------------
    for si in range(NSLAB):
        xs = slabs.pop(si)
        if si + 1 < NSLAB:
            slabs[si + 1] = load_slab(si + 1, 4)
        for tb in range(TB):
            psum = pox.tile([P, N], f32, name=f"{pfx}po{si}_{tb}", tag="po",
                            bufs=4)
            for kc in range(KC):
                nc.tensor.matmul(
                    psum[:],
                    xs[:, kc * TSLAB + tb * P: kc * TSLAB + (tb + 1) * P],
                    wp[:, kc * N:(kc + 1) * N],
                    start=(kc == 0),
                    stop=(kc == KC - 1),
                )
            osb = opool.tile([P, N], f32, name=f"{pfx}o{si}_{tb}", tag="o",
                             bufs=4)
            nc.scalar.copy(osb[:], psum[:])
            nc.scalar.dma_start(out_ap[ts(si * TB + tb, P), :], osb[:])


def build_nc(T, K, N, mode="bf16", reps=1):
    import concourse.tile as tile
    from concourse import bacc, mybir

    nc = bacc.Bacc(
        "TRN2",
        target_bir_lowering=False,
        debug=False,
        enable_asserts=False,
    )
    f32 = mybir.dt.float32
    in_dt = mybir.dt.bfloat16 if mode == "bf16" else f32
    xT_t = nc.dram_tensor("xT", [K, T], in_dt, kind="ExternalInput")
    wT_t = nc.dram_tensor("wT", [K, N], in_dt, kind="ExternalInput")
    wsc_t = nc.dram_tensor("wscale", [1, N], f32, kind="ExternalInput")
    wbi_t = nc.dram_tensor("wbias", [1, N], f32, kind="ExternalInput")
    out_t = nc.dram_tensor("out", [T, N], f32, kind="ExternalOutput")

    with tile.TileContext(nc) as tc:
        with ExitStack() as ctx:
            pools = make_pools(ctx, tc)
            for r in range(reps):
                build_body(
                    pools,
                    tc,
                    out_t.ap(),
                    xT_t.ap(),
                    wT_t.ap(),
                    wsc_t.ap(),
                    wbi_t.ap(),
                    mode=mode,
                    pfx=f"r{r}_",
                )
    nc.compile()
    return nc


_NC_CACHE = {}
_LAST_RESULT = None


def _get_nc(T, K, N, mode, reps=1):
    key = (T, K, N, mode, reps)
    if key not in _NC_CACHE:
        _NC_CACHE[key] = build_nc(T, K, N, mode, reps)
    return _NC_CACHE[key]


def _make_in_maps(inputs, mode=None):
    import ml_dtypes

    mode = mode or os.environ.get("KERNEL_MODE", "bf16")
    in_np = ml_dtypes.bfloat16 if mode == "bf16" else np.float32
    x = np.asarray(inputs["x"], dtype=np.float32).reshape(B * S, DIN)
    weight = np.asarray(inputs["weight"], dtype=np.float32)
    wscale = np.asarray(inputs["wscale"], dtype=np.float32).reshape(-1)
    wbias = np.asarray(inputs["wbias"], dtype=np.float32).reshape(-1)

    # host-side layout prep: both matmul operands go down in [K, *] layout
    xT = x.T.astype(in_np, order="C")  # [DIN, T]
    wT = weight.T.astype(in_np, order="C")  # [DIN, DOUT]

    in_maps = []
    for c in range(N_CORES):
        sl = slice(c * N_SHARD, (c + 1) * N_SHARD)
        in_maps.append(
            {
                "xT": xT,
                "wT": np.ascontiguousarray(wT[:, sl]),
                "wscale": np.ascontiguousarray(wscale[sl]).reshape(1, N_SHARD),
                "wbias": np.ascontiguousarray(wbias[sl]).reshape(1, N_SHARD),
            }
        )
    return in_maps


def kernel(x, weight, wscale, wbias):
    from concourse.bass_utils import run_bass_kernel_spmd

    mode = os.environ.get("KERNEL_MODE", "bf16")
    nc = _get_nc(B * S, DIN, N_SHARD, mode)
    in_maps = _make_in_maps(
        {"x": x, "weight": weight, "wscale": wscale, "wbias": wbias}, mode
    )

    trace = os.environ.get("KERNEL_TRACE", "0") == "1"
    res = run_bass_kernel_spmd(
        nc, in_maps, core_ids=list(range(N_CORES)), trace=trace
    )
    global _LAST_RESULT
    _LAST_RESULT = res
    if trace and res.exec_time_ns is not None:
        print(f"HW exec time: {res.exec_time_ns} ns")
    outs = [res.results[c]["out"] for c in range(N_CORES)]
    full = np.concatenate(outs, axis=1)  # [T, DOUT]
    return full.reshape(B, S, DOUT).astype(np.float32)


# revision 16
# speedup vs baseline: 32.3876x; 32.3876x over previous
"""Trainium2 Bass kernel for BinaryLinearWscales — transpose-free v2.

Math:  out = x @ (wscale * sign(weight) + wbias).T
     = x @ w''.T   with   w''[n,k] = wscale_n * sign(weight[n,k]) + wbias_n

Key ideas vs the v1 kernel:
  1. **No on-device transposes.**  The host passes x.T and weight.T
     (layout prep is part of the sharding step), so both matmul operands
     arrive in DRAM already in [K, *] layout.  v1 spent ~1024 PE
     transpose-mode ops (~275 ns each in-context, and transpose-mode
     does not count as PE-busy for the HAM clock gate) interleaved with
     its matmuls.
  2. **Scale and bias folded into the binary weight on-device** (w'' =
     wscale*sign(w) + wbias, computed once on DVE): no xsum
     ones-matmuls, no epilogue math — the PSUM result IS the output.
  3. **bf16 end-to-end.**  Host casts x.T / weight.T to bf16: halves HBM
     traffic (per-core DMA 44 MB vs 76 MB) and keeps the PE at
     1 col/cycle.  Measured rel err 3.2e-3 vs the 2e-2 gate.
  4. **Dense back-to-back matmul stream** (1024 MMs of N=512, nothing
     else on the PE) keeps HAM at K=8/8 (2.4 GHz).  Roofline: 1024 x
     512 cyc / 2.4 GHz = 218 us PE; DMA 44 MB / ~360 GB/s = 122 us.
     TimelineSim predicts 244.5 us/core.
  5. **Few, large DMAs**: x streams in 8 slabs of [4096k x 512t] bf16
     (4 MB), each as 4 batched 1 MB dma_starts via a 3D access pattern
     (p, kc, t) so a single InstDMACopy spans all 16 SDMA engines.
     x slabs ride the SP HWDGE ring; weights + outputs ride the ACT ring.

Sharding (tensor-parallel over DOUT): each of the 8 cores gets 512 rows
of weight/wscale/wbias and the full x; host concatenates core outputs
along the feature dim.

`reps`: number of back-to-back copies of the whole body inside one NEFF
— used by test.py to measure steady-state per-exec device time with the
axon dispatch round-trip cancelled ((t_reps - t_1)/(reps - 1)).
"""

import os
from contextlib import ExitStack

import numpy as np

P = 128

# full problem dims
B, S, DIN, DOUT = 2, 2048, 4096, 4096
N_CORES = 8
N_SHARD = DOUT // N_CORES  # 512

TSLAB = 512  # tokens per x slab

# experiment knobs (env-overridable for model scans)
MM_ORDER = os.environ.get("KERNEL_MM_ORDER", "kc")  # "kc" or "tb" outer
GP_START = int(os.environ.get("KERNEL_GP_START", "32"))  # 32 = all-DVE w''
SIGN_CHUNK = os.environ.get("KERNEL_SIGN_CHUNK", "1") == "1"
CONST_RING = os.environ.get("KERNEL_CONST_RING", "act")
POX_BUFS = int(os.environ.get("KERNEL_POX_BUFS", "8"))
COPY_ENG = os.environ.get("KERNEL_COPY_ENG", "vec")  # psum->sbuf copy engine
OUT_RING = os.environ.get("KERNEL_OUT_RING", "act")
PRE_SPLIT = int(os.environ.get("KERNEL_PRE_SPLIT", "4"))


def make_pools(ctx, tc):
    return {
        "x": ctx.enter_context(tc.tile_pool(name="x", bufs=2)),
        "w": ctx.enter_context(tc.tile_pool(name="w", bufs=2)),
        "const": ctx.enter_context(tc.tile_pool(name="const", bufs=1)),
        "osb": ctx.enter_context(tc.tile_pool(name="osb", bufs=POX_BUFS)),
        "pox": ctx.enter_context(tc.tile_pool(name="pox", bufs=POX_BUFS, space="PSUM")),
    }


def build_body(pools, tc, out_ap, xT_ap, wT_ap, wscale_ap, wbias_ap,
               mode="bf16", pfx=""):
    import concourse.bass as bass
    from concourse import mybir
    from concourse.bass import ts

    nc = tc.nc
    K, T = xT_ap.shape
    K2, N = wT_ap.shape
    assert K == K2 and K % P == 0 and T % TSLAB == 0 and N <= 512
    KC = K // P  # 32 k chunks
    NSLAB = T // TSLAB  # 8
    TB = TSLAB // P  # 4

    f32 = mybir.dt.float32
    bf16 = mybir.dt.bfloat16
    f32r = mybir.dt.float32r
    Alu = mybir.AluOpType
    mm_dt = bf16 if mode == "bf16" else f32r

    xpool, wpool, const, opool, pox = (
        pools["x"], pools["w"], pools["const"], pools["osb"], pools["pox"],
    )

    xT3 = xT_ap.rearrange("(kc p) t -> p kc t", p=P)  # [128, KC, T]

    def load_slab(si, split):
        """One x slab = [K, TSLAB] tokens, flat SBUF layout [p, kc*TSLAB+t].

        Batched 3D dma_starts (1 MB each) hit near-peak HBM bandwidth and
        span all 16 SDMA engines per transfer."""
        xs = xpool.tile([P, KC * TSLAB], mm_dt, name=f"{pfx}xs{si}",
                        tag="xs", bufs=2)
        xs3 = xs[:].rearrange("p (kc t) -> p kc t", kc=KC)
        step = KC // split
        dma = nc.sync.dma_start if mode == "bf16" else nc.gpsimd.dma_start
        for d in range(split):
            dma(
                xs3[:, d * step:(d + 1) * step, :],
                xT3[:, d * step:(d + 1) * step, ts(si, TSLAB)],
            )
        return xs

    # x slab 0 first in program order, split fine so the PE starts early
    slabs = {0: load_slab(0, 8)}

    # ---------------- constants (staged via SP ring; W rides ACT) ----------
    wsc_stage = const.tile([1, N], f32, name=f"{pfx}wsc_stage", tag="wsc_stage")
    getattr(nc, "sync" if CONST_RING == "sync" else "scalar").dma_start(wsc_stage[:], wscale_ap[:, :])
    wbi_stage = const.tile([1, N], f32, name=f"{pfx}wbi_stage", tag="wbi_stage")
    getattr(nc, "sync" if CONST_RING == "sync" else "scalar").dma_start(wbi_stage[:], wbias_ap[:, :])
    # mm_dt copies for same-dtype DVE ops (precision loss is negligible:
    # w'' itself is rounded to mm_dt anyway)
    wsc_nar = const.tile([1, N], mm_dt, name=f"{pfx}wsc_nar", tag="wsc_nar")
    nc.vector.tensor_copy(wsc_nar[:], wsc_stage[:])
    wbi_nar = const.tile([1, N], mm_dt, name=f"{pfx}wbi_nar", tag="wbi_nar")
    nc.vector.tensor_copy(wbi_nar[:], wbi_stage[:])
    wscale_rep = const.tile([P, N], mm_dt, name=f"{pfx}wscale_rep",
                            tag="wscale_rep")
    nc.gpsimd.partition_broadcast(wscale_rep[:], wsc_nar[:])
    wbias_rep = const.tile([P, N], mm_dt, name=f"{pfx}wbias_rep",
                           tag="wbias_rep")
    nc.gpsimd.partition_broadcast(wbias_rep[:], wbi_nar[:])

    # ---------------- w'' = wscale*sign(w) + wbias, cached all kernel -------
    # One persistent SBUF tile [128, KC*N]; DMA'd in 1 MB chunks, signed and
    # scaled in WCHUNK-kc groups so the first matmuls start early.  The DVE
    # work (~26 us serial) gates the first token-block's matmuls, so split
    # production across DVE (leading 2/3, consumed first) and GPSIMD
    # (trailing 1/3, ~2x slower per element but fully parallel).
    wp = wpool.tile([P, KC * N], mm_dt, name=f"{pfx}wp", tag="wp", bufs=2)
    wp3 = wp[:].rearrange("p (kc n) -> p kc n", kc=KC)
    wT3 = wT_ap.rearrange("(kc p) n -> p kc n", p=P)
    WCHUNK = 4  # kc per production chunk (0.5 MB W DMA pieces)
    wdma = nc.scalar.dma_start if mode == "bf16" else nc.gpsimd.dma_start
    for c in range(KC // WCHUNK):
        sl = slice(c * WCHUNK, (c + 1) * WCHUNK)
        wdma(wp3[:, sl, :], wT3[:, sl, :])
    for c in range(KC // WCHUNK):
        lo, hi = c * WCHUNK, (c + 1) * WCHUNK
        if SIGN_CHUNK:
            seg = wp[:, lo * N:hi * N]
            nc.vector.tensor_scalar(
                out=seg, in0=seg, scalar1=0.0, scalar2=2.0,
                op0=Alu.is_ge, op1=Alu.mult,
            )
        for eng, kcs in (
            (nc.vector, [k for k in range(lo, hi) if k < GP_START]),
            (nc.gpsimd, [k for k in range(lo, hi) if k >= GP_START]),
        ):
            for kc in kcs:
                wk = wp[:, kc * N:(kc + 1) * N]
                if not SIGN_CHUNK:
                    # (w >= 0) * 2 -> {0, 2}
                    eng.tensor_scalar(
                        out=wk, in0=wk, scalar1=0.0, scalar2=2.0,
                        op0=Alu.is_ge, op1=Alu.mult,
                    )
                # ({0,2} - 1) * wscale -> +-wscale
                eng.scalar_tensor_tensor(
                    out=wk, in0=wk, scalar=-1.0, in1=wscale_rep[:],
                    op0=Alu.add, op1=Alu.mult,
                )
                # + wbias
                eng.tensor_add(wk, wk, wbias_rep[:])

    # ---------------- main phase: pure matmul stream ----------------
    # kc-outer / tb-inner: 4 concurrent PSUM accumulation groups per slab, so
    # the PE consumes each w'' chunk at ~0.85 us/kc — matched to the w''
    # production rate — instead of needing all 32 kc within the first 7 us.
    # pox bufs=8 (all 8 banks): the next slab's groups open while the
    # previous slab's drain through ACT copies.
    for si in range(NSLAB):
        xs = slabs.pop(si)
        if si + 1 < NSLAB:
            slabs[si + 1] = load_slab(si + 1, PRE_SPLIT)
        psums = [
            pox.tile([P, N], f32, name=f"{pfx}po{si}_{tb}", tag="po", bufs=8)
            for tb in range(TB)
        ]
        order = (
            [(kc, tb) for kc in range(KC) for tb in range(TB)]
            if MM_ORDER == "kc"
            else [(kc, tb) for tb in range(TB) for kc in range(KC)]
        )
        for kc, tb in order:
            nc.tensor.matmul(
                psums[tb][:],
                xs[:, kc * TSLAB + tb * P: kc * TSLAB + (tb + 1) * P],
                wp[:, kc * N:(kc + 1) * N],
                start=(kc == 0),
                stop=(kc == KC - 1),
            )
        for tb in range(TB):
            osb = opool.tile([P, N], f32, name=f"{pfx}o{si}_{tb}", tag="o",
                             bufs=POX_BUFS)
            if COPY_ENG == "act":
                nc.scalar.copy(osb[:], psums[tb][:])
            else:
                nc.vector.tensor_copy(osb[:], psums[tb][:])
            getattr(nc, "sync" if OUT_RING == "sync" else "scalar").dma_start(
                out_ap[ts(si * TB + tb, P), :], osb[:])


def build_nc(T, K, N, mode="bf16", reps=1):
    import concourse.tile as tile
    from concourse import bacc, mybir

    nc = bacc.Bacc(
        "TRN2",
        target_bir_lowering=False,
        debug=False,
        enable_asserts=False,
    )
    f32 = mybir.dt.float32
    in_dt = mybir.dt.bfloat16 if mode == "bf16" else f32
    xT_t = nc.dram_tensor("xT", [K, T], in_dt, kind="ExternalInput")
    wT_t = nc.dram_tensor("wT", [K, N], in_dt, kind="ExternalInput")
    wsc_t = nc.dram_tensor("wscale", [1, N], f32, kind="ExternalInput")
    wbi_t = nc.dram_tensor("wbias", [1, N], f32, kind="ExternalInput")
    out_t = nc.dram_tensor("out", [T, N], f32, kind="ExternalOutput")

    with tile.TileContext(nc) as tc:
        with ExitStack() as ctx:
            pools = make_pools(ctx, tc)
            for r in range(reps):
                build_body(
                    pools,
                    tc,
                    out_t.ap(),
                    xT_t.ap(),
                    wT_t.ap(),
                    wsc_t.ap(),
                    wbi_t.ap(),
                    mode=mode,
                    pfx=f"r{r}_",
                )
    nc.compile()
    return nc


_NC_CACHE = {}
_LAST_RESULT = None


def _get_nc(T, K, N, mode, reps=1):
    key = (T, K, N, mode, reps)
    if key not in _NC_CACHE:
        _NC_CACHE[key] = build_nc(T, K, N, mode, reps)
    return _NC_CACHE[key]


def _make_in_maps(inputs, mode=None):
    import ml_dtypes

    mode = mode or os.environ.get("KERNEL_MODE", "bf16")
    in_np = ml_dtypes.bfloat16 if mode == "bf16" else np.float32
    x = np.asarray(inputs["x"], dtype=np.float32).reshape(B * S, DIN)
    weight = np.asarray(inputs["weight"], dtype=np.float32)
    wscale = np.asarray(inputs["wscale"], dtype=np.float32).reshape(-1)
    wbias = np.asarray(inputs["wbias"], dtype=np.float32).reshape(-1)

    # host-side layout prep: both matmul operands go down in [K, *] layout
    xT = x.T.astype(in_np, order="C")  # [DIN, T]
    wT = weight.T.astype(in_np, order="C")  # [DIN, DOUT]

    in_maps = []
    for c in range(N_CORES):
        sl = slice(c * N_SHARD, (c + 1) * N_SHARD)
        in_maps.append(
            {
                "xT": xT,
                "wT": np.ascontiguousarray(wT[:, sl]),
                "wscale": np.ascontiguousarray(wscale[sl]).reshape(1, N_SHARD),
                "wbias": np.ascontiguousarray(wbias[sl]).reshape(1, N_SHARD),
            }
        )
    return in_maps


def kernel(x, weight, wscale, wbias):
    from concourse.bass_utils import run_bass_kernel_spmd

    mode = os.environ.get("KERNEL_MODE", "bf16")
    nc = _get_nc(B * S, DIN, N_SHARD, mode)
    in_maps = _make_in_maps(
        {"x": x, "weight": weight, "wscale": wscale, "wbias": wbias}, mode
    )

    trace = os.environ.get("KERNEL_TRACE", "0") == "1"
    res = run_bass_kernel_spmd(
        nc, in_maps, core_ids=list(range(N_CORES)), trace=trace
    )
    global _LAST_RESULT
    _LAST_RESULT = res
    if trace and res.exec_time_ns is not None:
        print(f"HW exec time: {res.exec_time_ns} ns")
    outs = [res.results[c]["out"] for c in range(N_CORES)]
    full = np.concatenate(outs, axis=1)  # [T, DOUT]
    return full.reshape(B, S, DOUT).astype(np.float32)


# revision 18
# speedup vs baseline: 39.4795x; 1.2190x over previous
"""Trainium2 Bass kernel for BinaryLinearWscales — transpose-free v2.

Math:  out = x @ (wscale * sign(weight) + wbias).T
     = x @ w''.T   with   w''[n,k] = wscale_n * sign(weight[n,k]) + wbias_n

Key ideas vs the v1 kernel:
  1. **No on-device transposes.**  The host passes x.T and weight.T
     (layout prep is part of the sharding step), so both matmul operands
     arrive in DRAM already in [K, *] layout.  v1 spent ~1024 PE
     transpose-mode ops (~275 ns each in-context, and transpose-mode
     does not count as PE-busy for the HAM clock gate) interleaved with
     its matmuls.
  2. **Scale and bias folded into the binary weight on-device** (w'' =
     wscale*sign(w) + wbias, computed once on DVE): no xsum
     ones-matmuls, no epilogue math — the PSUM result IS the output.
  3. **bf16 end-to-end.**  Host casts x.T / weight.T to bf16: halves HBM
     traffic (per-core DMA 44 MB vs 76 MB) and keeps the PE at
     1 col/cycle.  Measured rel err 3.2e-3 vs the 2e-2 gate.
  4. **Dense back-to-back matmul stream** (1024 MMs of N=512, nothing
     else on the PE) keeps HAM at K=8/8 (2.4 GHz).  Roofline: 1024 x
     512 cyc / 2.4 GHz = 218 us PE; DMA 44 MB / ~360 GB/s = 122 us.
     TimelineSim predicts 244.5 us/core.
  5. **Few, large DMAs**: x streams in 8 slabs of [4096k x 512t] bf16
     (4 MB), each as 4 batched 1 MB dma_starts via a 3D access pattern
     (p, kc, t) so a single InstDMACopy spans all 16 SDMA engines.
     x slabs ride the SP HWDGE ring; weights + outputs ride the ACT ring.

Sharding (tensor-parallel over DOUT): each of the 8 cores gets 512 rows
of weight/wscale/wbias and the full x; host concatenates core outputs
along the feature dim.

`reps`: number of back-to-back copies of the whole body inside one NEFF
— used by test.py to measure steady-state per-exec device time with the
axon dispatch round-trip cancelled ((t_reps - t_1)/(reps - 1)).
"""

import os
from contextlib import ExitStack

import numpy as np

P = 128

# full problem dims
B, S, DIN, DOUT = 2, 2048, 4096, 4096
N_CORES = 8
N_SHARD = DOUT // N_CORES  # 512

TSLAB = 512  # tokens per x slab

# experiment knobs (env-overridable for model scans)
MM_ORDER = os.environ.get("KERNEL_MM_ORDER", "kc")  # "kc" or "tb" outer
GP_START = int(os.environ.get("KERNEL_GP_START", "32"))  # 32 = all-DVE w''
SIGN_CHUNK = os.environ.get("KERNEL_SIGN_CHUNK", "1") == "1"
CONST_RING = os.environ.get("KERNEL_CONST_RING", "act")
POX_BUFS = int(os.environ.get("KERNEL_POX_BUFS", "8"))
COPY_ENG = os.environ.get("KERNEL_COPY_ENG", "vec")  # psum->sbuf copy engine
OUT_RING = os.environ.get("KERNEL_OUT_RING", "act")
PRE_SPLIT = int(os.environ.get("KERNEL_PRE_SPLIT", "4"))
W_RING = os.environ.get("KERNEL_W_RING", "act")  # gpsimd SWDGE keeps W off the ACT ring


def make_pools(ctx, tc):
    return {
        "x": ctx.enter_context(tc.tile_pool(name="x", bufs=2)),
        "w": ctx.enter_context(tc.tile_pool(name="w", bufs=2)),
        "const": ctx.enter_context(tc.tile_pool(name="const", bufs=1)),
        "osb": ctx.enter_context(tc.tile_pool(name="osb", bufs=POX_BUFS)),
        "pox": ctx.enter_context(tc.tile_pool(name="pox", bufs=POX_BUFS, space="PSUM")),
    }


def build_body(pools, tc, out_ap, xT_ap, wT_ap, wscale_ap, wbias_ap,
               mode="bf16", pfx=""):
    import concourse.bass as bass
    from concourse import mybir
    from concourse.bass import ts

    nc = tc.nc
    K, T = xT_ap.shape
    K2, N = wT_ap.shape
    assert K == K2 and K % P == 0 and T % TSLAB == 0 and N <= 512
    KC = K // P  # 32 k chunks
    NSLAB = T // TSLAB  # 8
    TB = TSLAB // P  # 4

    f32 = mybir.dt.float32
    bf16 = mybir.dt.bfloat16
    f32r = mybir.dt.float32r
    Alu = mybir.AluOpType
    mm_dt = bf16 if mode == "bf16" else f32r

    xpool, wpool, const, opool, pox = (
        pools["x"], pools["w"], pools["const"], pools["osb"], pools["pox"],
    )

    xT3 = xT_ap.rearrange("(kc p) t -> p kc t", p=P)  # [128, KC, T]

    def load_slab(si, split):
        """One x slab = [K, TSLAB] tokens, flat SBUF layout [p, kc*TSLAB+t].

        Batched 3D dma_starts (1 MB each) hit near-peak HBM bandwidth and
        span all 16 SDMA engines per transfer."""
        xs = xpool.tile([P, KC * TSLAB], mm_dt, name=f"{pfx}xs{si}",
                        tag="xs", bufs=2)
        xs3 = xs[:].rearrange("p (kc t) -> p kc t", kc=KC)
        step = KC // split
        dma = nc.sync.dma_start if mode == "bf16" else nc.gpsimd.dma_start
        for d in range(split):
            dma(
                xs3[:, d * step:(d + 1) * step, :],
                xT3[:, d * step:(d + 1) * step, ts(si, TSLAB)],
            )
        return xs

    # x slab 0 first in program order, split fine so the PE starts early
    slabs = {0: load_slab(0, 8)}

    # ---------------- constants (staged via SP ring; W rides ACT) ----------
    wsc_stage = const.tile([1, N], f32, name=f"{pfx}wsc_stage", tag="wsc_stage")
    getattr(nc, "sync" if CONST_RING == "sync" else "scalar").dma_start(wsc_stage[:], wscale_ap[:, :])
    wbi_stage = const.tile([1, N], f32, name=f"{pfx}wbi_stage", tag="wbi_stage")
    getattr(nc, "sync" if CONST_RING == "sync" else "scalar").dma_start(wbi_stage[:], wbias_ap[:, :])
    # mm_dt copies for same-dtype DVE ops (precision loss is negligible:
    # w'' itself is rounded to mm_dt anyway)
    wsc_nar = const.tile([1, N], mm_dt, name=f"{pfx}wsc_nar", tag="wsc_nar")
    nc.vector.tensor_copy(wsc_nar[:], wsc_stage[:])
    wbi_nar = const.tile([1, N], mm_dt, name=f"{pfx}wbi_nar", tag="wbi_nar")
    nc.vector.tensor_copy(wbi_nar[:], wbi_stage[:])
    wscale_rep = const.tile([P, N], mm_dt, name=f"{pfx}wscale_rep",
                            tag="wscale_rep")
    nc.gpsimd.partition_broadcast(wscale_rep[:], wsc_nar[:])
    wbias_rep = const.tile([P, N], mm_dt, name=f"{pfx}wbias_rep",
                           tag="wbias_rep")
    nc.gpsimd.partition_broadcast(wbias_rep[:], wbi_nar[:])

    # ---------------- w'' = wscale*sign(w) + wbias, cached all kernel -------
    # One persistent SBUF tile [128, KC*N]; DMA'd in 1 MB chunks, signed and
    # scaled in WCHUNK-kc groups so the first matmuls start early.  The DVE
    # work (~26 us serial) gates the first token-block's matmuls, so split
    # production across DVE (leading 2/3, consumed first) and GPSIMD
    # (trailing 1/3, ~2x slower per element but fully parallel).
    wp = wpool.tile([P, KC * N], mm_dt, name=f"{pfx}wp", tag="wp", bufs=2)
    wp3 = wp[:].rearrange("p (kc n) -> p kc n", kc=KC)
    wT3 = wT_ap.rearrange("(kc p) n -> p kc n", p=P)
    WCHUNK = 4  # kc per production chunk (0.5 MB W DMA pieces)
    wdma = (
        (nc.gpsimd.dma_start if W_RING == "gp" else nc.scalar.dma_start)
        if mode == "bf16" else nc.gpsimd.dma_start
    )
    for c in range(KC // WCHUNK):
        sl = slice(c * WCHUNK, (c + 1) * WCHUNK)
        wdma(wp3[:, sl, :], wT3[:, sl, :])
    for c in range(KC // WCHUNK):
        lo, hi = c * WCHUNK, (c + 1) * WCHUNK
        if SIGN_CHUNK:
            seg = wp[:, lo * N:hi * N]
            nc.vector.tensor_scalar(
                out=seg, in0=seg, scalar1=0.0, scalar2=2.0,
                op0=Alu.is_ge, op1=Alu.mult,
            )
        for eng, kcs in (
            (nc.vector, [k for k in range(lo, hi) if k < GP_START]),
            (nc.gpsimd, [k for k in range(lo, hi) if k >= GP_START]),
        ):
            for kc in kcs:
                wk = wp[:, kc * N:(kc + 1) * N]
                if not SIGN_CHUNK:
                    # (w >= 0) * 2 -> {0, 2}
                    eng.tensor_scalar(
                        out=wk, in0=wk, scalar1=0.0, scalar2=2.0,
                        op0=Alu.is_ge, op1=Alu.mult,
                    )
                # ({0,2} - 1) * wscale -> +-wscale
                eng.scalar_tensor_tensor(
                    out=wk, in0=wk, scalar=-1.0, in1=wscale_rep[:],
                    op0=Alu.add, op1=Alu.mult,
                )
                # + wbias
                eng.tensor_add(wk, wk, wbias_rep[:])

    # ---------------- main phase: pure matmul stream ----------------
    # kc-outer / tb-inner: 4 concurrent PSUM accumulation groups per slab, so
    # the PE consumes each w'' chunk at ~0.85 us/kc — matched to the w''
    # production rate — instead of needing all 32 kc within the first 7 us.
    # pox bufs=8 (all 8 banks): the next slab's groups open while the
    # previous slab's drain through ACT copies.
    for si in range(NSLAB):
        xs = slabs.pop(si)
        if si + 1 < NSLAB:
            slabs[si + 1] = load_slab(si + 1, PRE_SPLIT)
        psums = [
            pox.tile([P, N], f32, name=f"{pfx}po{si}_{tb}", tag="po", bufs=8)
            for tb in range(TB)
        ]
        order = (
            [(kc, tb) for kc in range(KC) for tb in range(TB)]
            if MM_ORDER == "kc"
            else [(kc, tb) for tb in range(TB) for kc in range(KC)]
        )
        for kc, tb in order:
            nc.tensor.matmul(
                psums[tb][:],
                xs[:, kc * TSLAB + tb * P: kc * TSLAB + (tb + 1) * P],
                wp[:, kc * N:(kc + 1) * N],
                start=(kc == 0),
                stop=(kc == KC - 1),
            )
        for tb in range(TB):
            osb = opool.tile([P, N], f32, name=f"{pfx}o{si}_{tb}", tag="o",
                             bufs=POX_BUFS)
            if COPY_ENG == "act":
                nc.scalar.copy(osb[:], psums[tb][:])
            else:
                nc.vector.tensor_copy(osb[:], psums[tb][:])
            getattr(nc, "sync" if OUT_RING == "sync" else "scalar").dma_start(
                out_ap[ts(si * TB + tb, P), :], osb[:])


def build_nc(T, K, N, mode="bf16", reps=1):
    import concourse.tile as tile
    from concourse import bacc, mybir

    nc = bacc.Bacc(
        "TRN2",
        target_bir_lowering=False,
        debug=False,
        enable_asserts=False,
    )
    f32 = mybir.dt.float32
    in_dt = mybir.dt.bfloat16 if mode == "bf16" else f32
    xT_t = nc.dram_tensor("xT", [K, T], in_dt, kind="ExternalInput")
    wT_t = nc.dram_tensor("wT", [K, N], in_dt, kind="ExternalInput")
    wsc_t = nc.dram_tensor("wscale", [1, N], f32, kind="ExternalInput")
    wbi_t = nc.dram_tensor("wbias", [1, N], f32, kind="ExternalInput")
    out_t = nc.dram_tensor("out", [T, N], f32, kind="ExternalOutput")

    with tile.TileContext(nc) as tc:
        with ExitStack() as ctx:
            pools = make_pools(ctx, tc)
            for r in range(reps):
                build_body(
                    pools,
                    tc,
                    out_t.ap(),
                    xT_t.ap(),
                    wT_t.ap(),
                    wsc_t.ap(),
                    wbi_t.ap(),
                    mode=mode,
                    pfx=f"r{r}_",
                )
    nc.compile()
    return nc


_NC_CACHE = {}
_LAST_RESULT = None


def _get_nc(T, K, N, mode, reps=1):
    key = (T, K, N, mode, reps)
    if key not in _NC_CACHE:
        _NC_CACHE[key] = build_nc(T, K, N, mode, reps)
    return _NC_CACHE[key]


def _make_in_maps(inputs, mode=None):
    import ml_dtypes

    mode = mode or os.environ.get("KERNEL_MODE", "bf16")
    in_np = ml_dtypes.bfloat16 if mode == "bf16" else np.float32
    x = np.asarray(inputs["x"], dtype=np.float32).reshape(B * S, DIN)
    weight = np.asarray(inputs["weight"], dtype=np.float32)
    wscale = np.asarray(inputs["wscale"], dtype=np.float32).reshape(-1)
    wbias = np.asarray(inputs["wbias"], dtype=np.float32).reshape(-1)

    # host-side layout prep: both matmul operands go down in [K, *] layout
    xT = x.T.astype(in_np, order="C")  # [DIN, T]
    wT = weight.T.astype(in_np, order="C")  # [DIN, DOUT]

    in_maps = []
    for c in range(N_CORES):
        sl = slice(c * N_SHARD, (c + 1) * N_SHARD)
        in_maps.append(
            {
                "xT": xT,
                "wT": np.ascontiguousarray(wT[:, sl]),
                "wscale": np.ascontiguousarray(wscale[sl]).reshape(1, N_SHARD),
                "wbias": np.ascontiguousarray(wbias[sl]).reshape(1, N_SHARD),
            }
        )
    return in_maps


def kernel(x, weight, wscale, wbias):
    from concourse.bass_utils import run_bass_kernel_spmd

    mode = os.environ.get("KERNEL_MODE", "bf16")
    nc = _get_nc(B * S, DIN, N_SHARD, mode)
    in_maps = _make_in_maps(
        {"x": x, "weight": weight, "wscale": wscale, "wbias": wbias}, mode
    )

    trace = os.environ.get("KERNEL_TRACE", "0") == "1"
    res = run_bass_kernel_spmd(
        nc, in_maps, core_ids=list(range(N_CORES)), trace=trace
    )
    global _LAST_RESULT
    _LAST_RESULT = res
    if trace and res.exec_time_ns is not None:
        print(f"HW exec time: {res.exec_time_ns} ns")
    outs = [res.results[c]["out"] for c in range(N_CORES)]
    full = np.concatenate(outs, axis=1)  # [T, DOUT]
    return full.reshape(B, S, DOUT).astype(np.float32)
